# revision 11
# baseline (speedup 1.0000x reference)
"""Multi-head attention (B=2, L=2048, H=1024, NH=16) on 8 TRN2 NeuronCores.

Sharding: data-parallel over batch (2) x tensor-parallel over heads (4 groups
of 4 heads).  core = b*4 + g handles batch b, heads [4g, 4g+4).  Wq/Wk/Wv are
split column-wise, Wo row-wise; each core produces a partial [L, H] output
that the host sums per batch (the row-parallel all-reduce done host-side).

Device math (per core), all matmuls bf16 inputs / fp32 PSUM accumulation:
  QT = (Wq*0.125)^T x^T          [256, 2048]  (softmax scale folded into Wq)
  KT = Wk^T y^T                  [256, 2048]
  V  = y Wv                      [2048, 256]  (heads side by side, no ones)

The 4 local heads form 2 pairs (p = h//2); within a pair head A lives on
SBUF partitions 0-63 and head B on 64-127 of qT/kT.  Work is organized as
4 units = (pair, lq-chunk) x 32 slots = (lk-tile, 512-query half):

  S:     two K=64 matmuls (A rows 0-63 -> psS[:, 0:512], B rows 64-127 ->
         psS[:, 512:1024]) issued adjacent run as concurrent PE row tiles.
  exp:   one ScalarE ACTIVATE [128, 1024] psS -> pt bf16; 128 of these at
         ~1.33us are the pacing stream.
  O:     two M=64 col-tiled matmuls (A -> psO[0:64, sl], B -> psO[64:128,
         sl]) accumulating over the 16 lk tiles; concurrent col tiles.
  denom: per lk tile one 4-way col-tiled quad of M=1 ones-matmuls (A/B x
         sl0/sl1 -> psDen partitions 0/32/64/96) accumulates the softmax
         denominators in one PSUM bank.
  norm:  DVE recip + GpSimd partition-broadcast + DVE mul -> oT bf16.
  out:   out[lq, 1024] += O'^T_cat^T Wo  (partial; host sums the groups).

PSUM: psS 2x[128,1024]f32 (4 banks) + psO [128,1024]f32 (2) + psDen
[128,512]f32 (1) + psProj [128,512]f32 (1, projection/stage-3 groups) = 8.
Projections and stage-3 pieces ride the slot stream as single-group hooks; a
dummy-matmul stream during the input-DMA window holds the PE clock gate at
full rate; inputs are host-packed partition-major so every DMA is 128
contiguous runs (SP descriptor generation otherwise dominates startup).
"""

import numpy as np
import ml_dtypes

B, L, H, NH, D = 2, 2048, 1024, 16, 64
GP = 4            # head-groups (tensor-parallel factor)
CH = H // GP      # 256 local projection cols per core
HL = NH // GP     # 4 local heads
LQ = 1024         # lq chunk size
NLQ = L // LQ
NKT = L // 128    # 16 lk tiles
BF16 = ml_dtypes.bfloat16

_CACHE = {}


def _build():
    import concourse.mybir as mybir
    import concourse.tile as tile
    from concourse import bacc

    dt = mybir.dt
    f32, bf16 = dt.float32, dt.bfloat16
    Exp = mybir.ActivationFunctionType.Exp

    nc = bacc.Bacc("TRN2", target_bir_lowering=False, debug=False)
    # all inputs host-packed partition-major so each DMA is 128 long
    # contiguous runs
    xT = nc.declare_dram_parameter("xT", [128, NLQ, 2, 8, 512], bf16,
                                   isOutput=False)
    yT = nc.declare_dram_parameter("yT", [128, NLQ, 2, 8, 512], bf16,
                                   isOutput=False)
    wq = nc.declare_dram_parameter("wq", [128, 8, CH], bf16, isOutput=False)
    wk = nc.declare_dram_parameter("wk", [128, 8, CH], bf16, isOutput=False)
    wv = nc.declare_dram_parameter("wv", [128, 8, CH], bf16, isOutput=False)
    wo = nc.declare_dram_parameter("wo", [128, 2, H], bf16, isOutput=False)
    out = nc.declare_dram_parameter("out", [L, H], f32, isOutput=True)

    UNITS = [(0, 0), (1, 0), (0, 1), (1, 1)]  # (pair, chunk)

    with tile.TileContext(nc) as tc:
        with (
            tc.tile_pool(name="w", bufs=1) as wpool,
            tc.tile_pool(name="acts", bufs=1) as apool,
            tc.tile_pool(name="psS", bufs=2, space="PSUM") as psSp,
            tc.tile_pool(name="psO", bufs=1, space="PSUM") as psOp,
            tc.tile_pool(name="psD", bufs=1, space="PSUM") as psDp,
            tc.tile_pool(name="psP", bufs=1, space="PSUM") as psPp,
            tc.tile_pool(name="pt", bufs=7) as ptpool,
            tc.tile_pool(name="oT", bufs=2) as otpool,
            tc.tile_pool(name="sm", bufs=4) as smpool,
            tc.tile_pool(name="ocp", bufs=2) as ocppool,
            tc.tile_pool(name="osb", bufs=4) as opool,
        ):
            # prefetch the exp activation table while input DMAs run
            dummy = smpool.tile([1, 8], f32, tag="dummy")
            nc.vector.memset(dummy, 0.0)
            nc.scalar.activation(dummy, dummy, Exp)

            # ones column (denominator matmuls) + junk rhs (PE warm-up)
            ones_sb = apool.tile([128, 1], bf16, tag="ones")
            nc.vector.memset(ones_sb, 1.0)
            junk_sb = apool.tile([128, 512], bf16, tag="junk")
            nc.vector.memset(junk_sb, 0.0)

            # ---- input DMAs: weights first (small), then activations in
            # 512-column chunks consumed by projection groups as they land -
            wk_sb = wpool.tile([128, 8, CH], bf16, tag="wk")
            nc.sync.dma_start(wk_sb, wk[:, :, :])
            wv_sb = wpool.tile([128, 8, CH], bf16, tag="wv")
            nc.sync.dma_start(wv_sb, wv[:, :, :])
            wq_sb = wpool.tile([128, 8, CH], bf16, tag="wq")
            nc.sync.dma_start(wq_sb, wq[:, :, :])
            yT_sb = apool.tile([128, NLQ, 2, 8, 512], bf16, tag="yT")
            xT_sb = apool.tile([128, NLQ, 2, 8, 512], bf16, tag="xT")
            nc.sync.dma_start(yT_sb[:, 0, 0], yT[:, 0, 0])
            nc.sync.dma_start(xT_sb[:, 0, 0], xT[:, 0, 0])
            nc.sync.dma_start(xT_sb[:, 0, 1], xT[:, 0, 1])
            nc.sync.dma_start(yT_sb[:, 0, 1], yT[:, 0, 1])
            for sl in range(2):
                nc.sync.dma_start(yT_sb[:, 1, sl], yT[:, 1, sl])
            for sl in range(2):
                nc.sync.dma_start(xT_sb[:, 1, sl], xT[:, 1, sl])
            wo_sb = wpool.tile([128, 2, H], bf16, tag="wo")
            nc.sync.dma_start(wo_sb, wo[:, :, :])

            qT_sb = apool.tile([128, 2, L], bf16, tag="qT")
            kT_sb = apool.tile([128, 2, L], bf16, tag="kT")
            v_sb = apool.tile([128, NKT, CH], bf16, tag="v")

            # ---- PE warm-up: a stream of cheap M=1 matmuls spanning the
            # input-DMA window keeps the HAM activity monitor at K=8/8 so
            # the projections (and slot 0 onward) run at full clock --------
            warm = psDp.tile([128, 512], f32, tag="psD", name="warm")
            for _ in range(56):
                nc.tensor.matmul(warm[0:1, :], lhsT=ones_sb[:, 0:1],
                                 rhs=junk_sb, start=True, stop=True)

            def proj_group(w_sb, act_sb, dst, ct, lh, sl, pool=None):
                # dst[:, ct, lh*LQ+sl*512 : +512] via one 8-matmul group
                pool = pool or psPp
                ps = pool.tile([128, 512], f32,
                               tag="psS" if pool is psSp else "psP")
                for ht in range(8):
                    nc.tensor.matmul(
                        ps[:, 0:512],
                        lhsT=w_sb[:, ht, ct * 128:(ct + 1) * 128],
                        rhs=act_sb[:, lh, sl, ht, :],
                        start=(ht == 0), stop=(ht == 7),
                    )
                off = lh * LQ + sl * 512
                nc.vector.tensor_copy(dst[:, ct, off:off + 512],
                                      ps[:, 0:512])

            def v_group(lkt, pool=None):
                # one lk tile of V[lk, 4*64] bf16 (heads side by side)
                pool = pool or psPp
                ps = pool.tile([128, CH], f32,
                               tag="psS" if pool is psSp else "psP")
                for ht in range(8):
                    nc.tensor.matmul(
                        ps[:, :CH],
                        lhsT=yT_sb[:, lkt // 8, (lkt % 8) // 4, ht,
                                   (lkt % 4) * 128:(lkt % 4 + 1) * 128],
                        rhs=wv_sb[:, ht, :],
                        start=(ht == 0), stop=(ht == 7),
                    )
                nc.vector.tensor_copy(v_sb[:, lkt, :], ps[:, :CH])

            def emit_S(p, ci, lkt, sl):
                # head pair as two adjacent K=64 row-tile matmuls
                ps = psSp.tile([128, 1024], f32, tag="psS")
                q0 = ci * LQ + sl * 512
                for hh in range(2):
                    nc.tensor.matmul(
                        ps[:, hh * 512:(hh + 1) * 512],
                        lhsT=kT_sb[64 * hh:64 * hh + 64, p,
                                   lkt * 128:(lkt + 1) * 128],
                        rhs=qT_sb[64 * hh:64 * hh + 64, p, q0:q0 + 512],
                        start=True, stop=True,
                    )
                return ps

            def emit_O(p, psO_t, pt_t, lkt, sl):
                # head pair as two M=64 col-tile matmuls, same psO bank
                for hh in range(2):
                    nc.tensor.matmul(
                        psO_t[64 * hh:64 * hh + 64,
                              sl * 512:(sl + 1) * 512],
                        lhsT=v_sb[:, lkt,
                                  128 * p + 64 * hh:128 * p + 64 * hh + 64],
                        rhs=pt_t[:, hh * 512:(hh + 1) * 512],
                        start=(lkt == 0), stop=(lkt == NKT - 1),
                    )

            def emit_den(psD_t, pt0, pt1, lkt):
                # softmax denominators: 4-way col-tiled M=1 ones-matmuls
                # (A/B x sl0/sl1 -> partitions 0/32/64/96), one quad per lkt
                for hh in range(2):
                    for sl, ptt in ((0, pt0), (1, pt1)):
                        pr = 64 * hh + 32 * sl
                        nc.tensor.matmul(
                            psD_t[pr:pr + 1, 0:512],
                            lhsT=ones_sb[:, 0:1],
                            rhs=ptt[:, hh * 512:(hh + 1) * 512],
                            start=(lkt == 0), stop=(lkt == NKT - 1),
                            tile_position=(0, pr),
                        )

            oT = [otpool.tile([128, 2, LQ], bf16, tag="oT", name=f"oT{i}")
                  for i in range(NLQ)]

            def evac_O(psO_t):
                # psO -> two base-0 [64, 1024] tiles (per head) so the
                # normalize muls satisfy the DVE same-base-partition rule
                ocp = [ocppool.tile([64, 1024], f32, tag="ocp",
                                    name=f"ocp{h}") for h in range(2)]
                for hh in range(2):
                    nc.vector.tensor_copy(ocp[hh],
                                          psO_t[64 * hh:64 * hh + 64, :])
                return ocp

            def norm_a(psD_t, hh, sl):
                # denominator -> reciprocal (DVE): must precede the next
                # unit's first denominator quad (psDen is single-buffered)
                pr = 64 * hh + 32 * sl
                sums = smpool.tile([1, 512], f32, tag="sums")
                nc.vector.tensor_copy(sums, psD_t[pr:pr + 1, 0:512])
                recip = smpool.tile([1, 512], f32, tag="recip")
                nc.vector.reciprocal_approx_fast(recip, sums)
                return recip

            def norm_b(ui, recip, ocp, hh, sl):
                # partition-broadcast + normalize mul -> oT bf16
                p, ci = UNITS[ui]
                bcast = smpool.tile([64, 512], f32, tag="bcast")
                nc.gpsimd.partition_broadcast(bcast, recip)
                nc.vector.tensor_mul(
                    oT[ci][64 * hh:64 * hh + 64, p,
                           sl * 512:(sl + 1) * 512],
                    ocp[hh][:, sl * 512:(sl + 1) * 512],
                    bcast)

            def s3_half(ci, mt, nt, act_copy=False):
                # out[ci*LQ+mt*128 : +128, nt*512 : +512]: contraction over
                # both pairs (kt) in one 2-matmul group + copy + store
                pso = psPp.tile([128, 512], f32, tag="psP")
                for kt in range(2):
                    nc.tensor.matmul(
                        pso[:, 0:512],
                        lhsT=oT[ci][:, kt, mt * 128:(mt + 1) * 128],
                        rhs=wo_sb[:, kt, nt * 512:(nt + 1) * 512],
                        start=(kt == 0), stop=(kt == 1),
                    )
                osb = opool.tile([128, 512], f32, tag="osb")
                if act_copy:
                    nc.scalar.copy(osb, pso)
                else:
                    nc.vector.tensor_copy(osb, pso)
                nc.sync.dma_start(
                    out[ci * LQ + mt * 128:ci * LQ + (mt + 1) * 128,
                        nt * 512:(nt + 1) * 512], osb)

            def s3_full(ci, mt, act_copy=False):
                # tail-only full-width piece through the (now idle) psS
                # pool: 2-deep pipelining instead of the serial psP chain
                pso = psSp.tile([128, 1024], f32, tag="psS")
                for kt in range(2):
                    for nt in range(2):
                        nc.tensor.matmul(
                            pso[:, nt * 512:(nt + 1) * 512],
                            lhsT=oT[ci][:, kt, mt * 128:(mt + 1) * 128],
                            rhs=wo_sb[:, kt, nt * 512:(nt + 1) * 512],
                            start=(kt == 0), stop=(kt == 1),
                        )
                osb = opool.tile([128, 1024], f32, tag="osb")
                if act_copy:
                    nc.scalar.copy(osb, pso)
                else:
                    nc.vector.tensor_copy(osb, pso)
                nc.sync.dma_start(
                    out[ci * LQ + mt * 128:ci * LQ + (mt + 1) * 128, :],
                    osb)

            # ---- hook schedule: per unit, slot -> list of thunks.  Each
            # psP group occupies 2 slots (group + copy through the single
            # buffer), so group starts are spaced 2 apart with deadlines:
            # V(j) copy by slot 2j (its O slot), K pieces by their S-emit.
            hooks = [dict() for _ in range(4)]

            def add_hook(ui, s, job):
                hooks[ui].setdefault(s, []).append(job)

            def pj(ct, lh, sl, w=None):
                w_sb, a_sb, d_sb = ((wk_sb, yT_sb, kT_sb) if w == "k"
                                    else (wq_sb, xT_sb, qT_sb))
                return lambda: proj_group(w_sb, a_sb, d_sb, ct, lh, sl)

            u0 = {0: lambda: v_group(4),
                  1: pj(0, 0, 1, "k"),       # kT p0 lk 512-1023, S-emit s=6
                  3: lambda: v_group(5),
                  5: lambda: v_group(6),
                  7: lambda: v_group(7),     # due s=14
                  9: pj(0, 1, 0, "k"),       # kT p0 lk 1024-1535, emit s=14
                  11: lambda: v_group(8),
                  13: lambda: v_group(9),
                  15: lambda: v_group(10),   # due s=20
                  17: pj(0, 1, 1, "k"),      # kT p0 lk 1536-2047, emit s=22
                  19: lambda: v_group(11),   # due s=22
                  21: lambda: v_group(12),
                  23: lambda: v_group(13),
                  25: lambda: v_group(14),
                  26: lambda: v_group(15),   # due s=30
                  28: pj(1, 0, 0),           # qT p1 q 0-511, due unit1 s=0
                  29: pj(1, 0, 0, "k"),      # kT p1 lk 0-511, due unit1 s=0
                  30: pj(1, 0, 1)}           # qT p1 q 512-1023, due u1 s=1
            u1 = {4: pj(1, 0, 1, "k"),       # kT p1 lk 512-1023, emit s=6
                  10: pj(1, 1, 0, "k"),
                  18: pj(1, 1, 1, "k"),
                  24: pj(0, 1, 0),           # qT p0 q 1024-1535, due u2 s=0
                  28: pj(0, 1, 1)}
            u2 = {8: pj(1, 1, 0),            # due unit3, plenty of slack
                  10: pj(1, 1, 1)}
            for ui, table in ((0, u0), (1, u1), (2, u2)):
                for s, job in table.items():
                    add_hook(ui, s, job)
            # chunk-0 stage 3: oT[0] final once unit-1's norm-b lands
            # (unit-2 slots 4-7); sl0 rows first, then sl1 rows
            for i, (mt, nt) in enumerate((m, n) for m in range(8)
                                         for n in range(2)):
                add_hook(2, 12 + i, lambda m=mt, n=nt: s3_half(0, m, n))

            # ---- startup: what slot 0 strictly needs runs through the
            # psS pool (no other user yet, so it pipelines 2-deep); the
            # first six V tiles ride the psP chain inside the DMA window --
            proj_group(wk_sb, yT_sb, kT_sb, 0, 0, 0, pool=psSp)
            proj_group(wq_sb, xT_sb, qT_sb, 0, 0, 0, pool=psSp)
            proj_group(wq_sb, xT_sb, qT_sb, 0, 0, 1, pool=psSp)
            for j in range(4):
                v_group(j)
            state = {(0, 0): emit_S(0, 0, 0, 0), (0, 1): emit_S(0, 0, 0, 1)}

            # ---- main loop: 4 units x 32 slots --------------------------
            psO_prev = psD_prev = None
            for ui in range(4):
                p, ci = UNITS[ui]
                psO_t = psOp.tile([128, 1024], f32, tag="psO")
                psD_t = psDp.tile([128, 512], f32, tag="psD")
                pts = {}
                for s in range(32):
                    ptt = ptpool.tile([128, 1024], bf16, tag="pt")
                    nc.scalar.activation(ptt, state.pop((ui, s)), Exp)
                    pts[s] = ptt
                    t = s + 2
                    if t < 32:
                        state[(ui, t)] = emit_S(p, ci, t // 2, t % 2)
                    elif ui + 1 < 4:
                        np_, nci = UNITS[ui + 1]
                        state[(ui + 1, t - 32)] = emit_S(
                            np_, nci, (t - 32) // 2, (t - 32) % 2)
                    if s >= 1:
                        emit_O(p, psO_t, pts[s - 1], (s - 1) // 2,
                               (s - 1) % 2)
                    # denominator quads lag 3 lk tiles so the previous
                    # unit's psDen drain (slots 0-3) finishes first
                    if s >= 6 and s % 2 == 0:
                        k = (s - 6) // 2
                        emit_den(psD_t, pts[2 * k], pts[2 * k + 1], k)
                        del pts[2 * k], pts[2 * k + 1]
                    elif s == 31:
                        emit_den(psD_t, pts[26], pts[27], 13)
                        del pts[26], pts[27]
                    # previous unit's normalize: psO evacuation is the
                    # only DVE work in slots 0-1 (it gates this unit's
                    # first O matmul); recips land in slots 2-3 (they gate
                    # the first denominator quad at slot 6); broadcast+mul
                    # pieces follow in slots 4-7
                    if psO_prev is not None and s == 0:
                        ocp = evac_O(psO_prev)
                        rcell = {}
                        for i, (hh, sl) in enumerate(
                                ((0, 0), (1, 0), (0, 1), (1, 1))):
                            add_hook(ui, 2 + i // 2,
                                     lambda pd=psD_prev, h=hh, ss=sl,
                                     rc=rcell: rc.__setitem__(
                                         (h, ss), norm_a(pd, h, ss)))
                            add_hook(ui, 4 + i,
                                     lambda u=ui - 1, oc=ocp, h=hh, ss=sl,
                                     rc=rcell:
                                     norm_b(u, rc[(h, ss)], oc, h, ss))
                    for job in hooks[ui].get(s, ()):
                        job()
                emit_O(p, psO_t, pts[31], NKT - 1, 1)
                emit_den(psD_t, pts[28], pts[29], 14)
                emit_den(psD_t, pts[30], pts[31], 15)
                psO_prev, psD_prev = psO_t, psD_t

            # ---- tail: unit-3 normalize + chunk-1 stage 3 (full-width
            # pieces, 2-deep through the now-idle psS pool, copies
            # alternating between ScalarE and VectorE) --------------------
            ocp = evac_O(psO_prev)
            recips = {(hh, sl): norm_a(psD_prev, hh, sl)
                      for hh in range(2) for sl in range(2)}
            for hh in range(2):
                norm_b(3, recips[(hh, 0)], ocp, hh, 0)
            for mt in range(4):
                s3_full(1, mt, act_copy=(mt % 2 == 0))
            for hh in range(2):
                norm_b(3, recips[(hh, 1)], ocp, hh, 1)
            for mt in range(4, 8):
                s3_full(1, mt, act_copy=(mt % 2 == 0))
    nc.compile()
    return nc


def _get_nc():
    if "nc" not in _CACHE:
        _CACHE["nc"] = _build()
    return _CACHE["nc"]


def _pack_pm(a, t):
    # [t*128, N] -> [128, t, N] partition-major
    return a.reshape(t, 128, -1).transpose(1, 0, 2)


def _pack_act(a):
    # x[b] [L, H] -> xT packed [128, NLQ(lh), 2(sl), 8(t), 512] bf16
    v = _pack_pm(np.ascontiguousarray(a.T), 8)          # [128, 8, L]
    v = v.reshape(128, 8, NLQ, 2, 512).transpose(0, 2, 3, 1, 4)
    return np.ascontiguousarray(v).astype(BF16)


def _in_maps(x, y, Wq, Wk, Wv, Wo):
    maps = []
    for core in range(8):
        b, g = core // GP, core % GP
        cs = slice(g * CH, (g + 1) * CH)
        maps.append({
            "xT": _pack_act(x[b]),
            "yT": _pack_act(y[b]),
            "wq": np.ascontiguousarray(
                _pack_pm(Wq[:, cs] * np.float32(0.125), 8)).astype(BF16),
            "wk": np.ascontiguousarray(_pack_pm(Wk[:, cs], 8)).astype(BF16),
            "wv": np.ascontiguousarray(_pack_pm(Wv[:, cs], 8)).astype(BF16),
            "wo": np.ascontiguousarray(_pack_pm(Wo[cs, :], 2)).astype(BF16),
        })
    return maps


def _install_ntff_hook():
    """Provide the antenv.axon_hooks shim missing from this container so
    run_bass_kernel_spmd(trace=True) can drive NTFF profiling via ctypes."""
    import sys
    import types
    try:
        from antenv.axon_hooks import get_axon_ntff_profile_hook  # noqa: F401
        return
    except ImportError:
        pass
    from trn_agent_boot.trn_boot import _ntff_profile_via_ctypes
    hook = _ntff_profile_via_ctypes("/opt/axon/libaxon_pjrt.so")
    mod = types.ModuleType("antenv.axon_hooks")
    mod.get_axon_ntff_profile_hook = lambda: hook
    mod.set_axon_ntff_profile_hook = lambda h: None
    sys.modules["antenv.axon_hooks"] = mod


def _run(inputs, trace=False):
    from concourse import bass_utils

    if trace:
        _install_ntff_hook()

    x, y, bias = inputs["x"], inputs["y"], inputs["bias"]
    if np.count_nonzero(np.asarray(bias)):
        raise NotImplementedError("nonzero attention bias not supported")
    nc = _get_nc()
    maps = _in_maps(np.asarray(x, np.float32), np.asarray(y, np.float32),
                    np.asarray(inputs["Wq"], np.float32),
                    np.asarray(inputs["Wk"], np.float32),
                    np.asarray(inputs["Wv"], np.float32),
                    np.asarray(inputs["Wo"], np.float32))
    res = bass_utils.run_bass_kernel_spmd(
        nc, maps, list(range(8)), trace=trace)
    out = np.zeros((B, L, H), np.float32)
    for core in range(8):
        out[core // GP] += res.results[core]["out"]
    return out, res


def kernel(**inputs):
    out, _ = _run(inputs, trace=False)
    return out


# revision 12
# speedup vs baseline: 1.1993x; 1.1993x over previous
"""Multi-head attention (B=2, L=2048, H=1024, NH=16) on 8 TRN2 NeuronCores.

Sharding: data-parallel over batch (2) x tensor-parallel over heads (4 groups
of 4 heads).  core = b*4 + g handles batch b, heads [4g, 4g+4).  Wq/Wk/Wv are
split column-wise, Wo row-wise; each core produces a partial [L, H] output
that the host sums per batch (the row-parallel all-reduce done host-side).

Device math (per core), all matmuls bf16 inputs / fp32 PSUM accumulation:
  QT = (Wq*0.125)^T x^T          [256, 2048]  (softmax scale folded into Wq)
  KT = Wk^T y^T                  [256, 2048]
  V  = y Wv                      [2048, 256]  (heads side by side, no ones)

The 4 local heads form 2 pairs (p = h//2); within a pair head A lives on
SBUF partitions 0-63 and head B on 64-127 of qT/kT.  Work is organized as
4 units = (pair, lq-chunk) x 32 slots = (lk-tile, 512-query half):

  S:     two K=64 matmuls (A rows 0-63 -> psS[:, 0:512], B rows 64-127 ->
         psS[:, 512:1024]) issued adjacent run as concurrent PE row tiles.
  exp:   one ScalarE ACTIVATE [128, 1024] psS -> pt bf16; 128 of these at
         ~1.33us are the pacing stream.
  O:     two M=64 col-tiled matmuls (A -> psO[0:64, sl], B -> psO[64:128,
         sl]) accumulating over the 16 lk tiles; concurrent col tiles.
  denom: per lk tile one 4-way col-tiled quad of M=1 ones-matmuls (A/B x
         sl0/sl1 -> psDen partitions 0/32/64/96) accumulates the softmax
         denominators in one PSUM bank.
  norm:  DVE recip + GpSimd partition-broadcast + DVE mul -> oT bf16.
  out:   out[lq, 1024] += O'^T_cat^T Wo  (partial; host sums the groups).

PSUM: psS 2x[128,1024]f32 (4 banks) + psO [128,1024]f32 (2) + psDen
[128,512]f32 (1) + psProj [128,512]f32 (1, projection/stage-3 groups) = 8.
Projections and stage-3 pieces ride the slot stream as single-group hooks; a
dummy-matmul stream during the input-DMA window holds the PE clock gate at
full rate; inputs are host-packed partition-major so every DMA is 128
contiguous runs (SP descriptor generation otherwise dominates startup).
"""

import numpy as np
import ml_dtypes

B, L, H, NH, D = 2, 2048, 1024, 16, 64
GP = 4            # head-groups (tensor-parallel factor)
CH = H // GP      # 256 local projection cols per core
HL = NH // GP     # 4 local heads
LQ = 1024         # lq chunk size
NLQ = L // LQ
NKT = L // 128    # 16 lk tiles
BF16 = ml_dtypes.bfloat16

_CACHE = {}


def _build():
    import concourse.mybir as mybir
    import concourse.tile as tile
    from concourse import bacc

    dt = mybir.dt
    f32, bf16 = dt.float32, dt.bfloat16
    Exp = mybir.ActivationFunctionType.Exp

    nc = bacc.Bacc("TRN2", target_bir_lowering=False, debug=False)
    # all inputs host-packed partition-major so each DMA is 128 long
    # contiguous runs
    xT = nc.declare_dram_parameter("xT", [128, NLQ, 2, 8, 512], bf16,
                                   isOutput=False)
    yT = nc.declare_dram_parameter("yT", [128, NLQ, 2, 8, 512], bf16,
                                   isOutput=False)
    wq = nc.declare_dram_parameter("wq", [128, 8, CH], bf16, isOutput=False)
    wk = nc.declare_dram_parameter("wk", [128, 8, CH], bf16, isOutput=False)
    wv = nc.declare_dram_parameter("wv", [128, 8, CH], bf16, isOutput=False)
    wo = nc.declare_dram_parameter("wo", [128, 2, H], bf16, isOutput=False)
    out = nc.declare_dram_parameter("out", [L, H], f32, isOutput=True)

    UNITS = [(0, 0), (1, 0), (0, 1), (1, 1)]  # (pair, chunk)

    with tile.TileContext(nc) as tc:
        with (
            tc.tile_pool(name="w", bufs=1) as wpool,
            tc.tile_pool(name="acts", bufs=1) as apool,
            tc.tile_pool(name="psS", bufs=2, space="PSUM") as psSp,
            tc.tile_pool(name="psO", bufs=1, space="PSUM") as psOp,
            tc.tile_pool(name="psD", bufs=1, space="PSUM") as psDp,
            tc.tile_pool(name="psP", bufs=1, space="PSUM") as psPp,
            tc.tile_pool(name="pt", bufs=7) as ptpool,
            tc.tile_pool(name="oT", bufs=2) as otpool,
            tc.tile_pool(name="sm", bufs=4) as smpool,
            tc.tile_pool(name="ocp", bufs=2) as ocppool,
            tc.tile_pool(name="osb", bufs=4) as opool,
        ):
            # prefetch the exp activation table while input DMAs run
            dummy = smpool.tile([1, 8], f32, tag="dummy")
            nc.vector.memset(dummy, 0.0)
            nc.scalar.activation(dummy, dummy, Exp)

            # ones column (denominator matmuls)
            ones_sb = apool.tile([128, 1], bf16, tag="ones")
            nc.vector.memset(ones_sb, 1.0)

            # ---- input DMAs: weights first (small), then activations in
            # 512-column chunks consumed by projection groups as they land -
            wk_sb = wpool.tile([128, 8, CH], bf16, tag="wk")
            nc.sync.dma_start(wk_sb, wk[:, :, :])
            wv_sb = wpool.tile([128, 8, CH], bf16, tag="wv")
            nc.sync.dma_start(wv_sb, wv[:, :, :])
            wq_sb = wpool.tile([128, 8, CH], bf16, tag="wq")
            nc.sync.dma_start(wq_sb, wq[:, :, :])
            yT_sb = apool.tile([128, NLQ, 2, 8, 512], bf16, tag="yT")
            xT_sb = apool.tile([128, NLQ, 2, 8, 512], bf16, tag="xT")
            nc.sync.dma_start(yT_sb[:, 0, 0], yT[:, 0, 0])
            nc.sync.dma_start(xT_sb[:, 0, 0], xT[:, 0, 0])
            nc.sync.dma_start(xT_sb[:, 0, 1], xT[:, 0, 1])
            nc.sync.dma_start(yT_sb[:, 0, 1], yT[:, 0, 1])
            for sl in range(2):
                nc.sync.dma_start(yT_sb[:, 1, sl], yT[:, 1, sl])
            for sl in range(2):
                nc.sync.dma_start(xT_sb[:, 1, sl], xT[:, 1, sl])
            wo_sb = wpool.tile([128, 2, H], bf16, tag="wo")
            nc.sync.dma_start(wo_sb, wo[:, :, :])

            qT_sb = apool.tile([128, 2, L], bf16, tag="qT")
            kT_sb = apool.tile([128, 2, L], bf16, tag="kT")
            v_sb = apool.tile([128, NKT, CH], bf16, tag="v")

            def proj_group(w_sb, act_sb, dst, ct, lh, sl, pool=None):
                # dst[:, ct, lh*LQ+sl*512 : +512] via one 8-matmul group
                pool = pool or psPp
                ps = pool.tile([128, 512], f32,
                               tag="psS" if pool is psSp else "psP")
                for ht in range(8):
                    nc.tensor.matmul(
                        ps[:, 0:512],
                        lhsT=w_sb[:, ht, ct * 128:(ct + 1) * 128],
                        rhs=act_sb[:, lh, sl, ht, :],
                        start=(ht == 0), stop=(ht == 7),
                    )
                off = lh * LQ + sl * 512
                nc.vector.tensor_copy(dst[:, ct, off:off + 512],
                                      ps[:, 0:512])

            def v_group(lkt, pool=None):
                # one lk tile of V[lk, 4*64] bf16 (heads side by side)
                pool = pool or psPp
                ps = pool.tile([128, CH], f32,
                               tag="psS" if pool is psSp else "psP")
                for ht in range(8):
                    nc.tensor.matmul(
                        ps[:, :CH],
                        lhsT=yT_sb[:, lkt // 8, (lkt % 8) // 4, ht,
                                   (lkt % 4) * 128:(lkt % 4 + 1) * 128],
                        rhs=wv_sb[:, ht, :],
                        start=(ht == 0), stop=(ht == 7),
                    )
                nc.vector.tensor_copy(v_sb[:, lkt, :], ps[:, :CH])

            def emit_S(p, ci, lkt, sl):
                # head pair as two adjacent K=64 row-tile matmuls
                ps = psSp.tile([128, 1024], f32, tag="psS")
                q0 = ci * LQ + sl * 512
                for hh in range(2):
                    nc.tensor.matmul(
                        ps[:, hh * 512:(hh + 1) * 512],
                        lhsT=kT_sb[64 * hh:64 * hh + 64, p,
                                   lkt * 128:(lkt + 1) * 128],
                        rhs=qT_sb[64 * hh:64 * hh + 64, p, q0:q0 + 512],
                        start=True, stop=True,
                    )
                return ps

            def emit_O(p, psO_t, pt_t, lkt, sl):
                # head pair as two M=64 col-tile matmuls, same psO bank
                for hh in range(2):
                    nc.tensor.matmul(
                        psO_t[64 * hh:64 * hh + 64,
                              sl * 512:(sl + 1) * 512],
                        lhsT=v_sb[:, lkt,
                                  128 * p + 64 * hh:128 * p + 64 * hh + 64],
                        rhs=pt_t[:, hh * 512:(hh + 1) * 512],
                        start=(lkt == 0), stop=(lkt == NKT - 1),
                    )

            def emit_den(psD_t, pt0, pt1, lkt):
                # softmax denominators: 4-way col-tiled M=1 ones-matmuls
                # (A/B x sl0/sl1 -> partitions 0/32/64/96), one quad per lkt
                for hh in range(2):
                    for sl, ptt in ((0, pt0), (1, pt1)):
                        pr = 64 * hh + 32 * sl
                        nc.tensor.matmul(
                            psD_t[pr:pr + 1, 0:512],
                            lhsT=ones_sb[:, 0:1],
                            rhs=ptt[:, hh * 512:(hh + 1) * 512],
                            start=(lkt == 0), stop=(lkt == NKT - 1),
                            tile_position=(0, pr),
                        )

            oT = [otpool.tile([128, 2, LQ], bf16, tag="oT", name=f"oT{i}")
                  for i in range(NLQ)]

            def evac_O(psO_t):
                # psO -> two base-0 [64, 1024] tiles (per head) so the
                # normalize muls satisfy the DVE same-base-partition rule
                ocp = [ocppool.tile([64, 1024], f32, tag="ocp",
                                    name=f"ocp{h}") for h in range(2)]
                for hh in range(2):
                    nc.vector.tensor_copy(ocp[hh],
                                          psO_t[64 * hh:64 * hh + 64, :])
                return ocp

            def norm_a(psD_t, hh, sl):
                # denominator -> reciprocal (DVE): must precede the next
                # unit's first denominator quad (psDen is single-buffered)
                pr = 64 * hh + 32 * sl
                sums = smpool.tile([1, 512], f32, tag="sums")
                nc.vector.tensor_copy(sums, psD_t[pr:pr + 1, 0:512])
                recip = smpool.tile([1, 512], f32, tag="recip")
                nc.vector.reciprocal_approx_fast(recip, sums)
                return recip

            def norm_b(ui, recip, ocp, hh, sl):
                # partition-broadcast + normalize mul -> oT bf16
                p, ci = UNITS[ui]
                bcast = smpool.tile([64, 512], f32, tag="bcast")
                nc.gpsimd.partition_broadcast(bcast, recip)
                nc.vector.tensor_mul(
                    oT[ci][64 * hh:64 * hh + 64, p,
                           sl * 512:(sl + 1) * 512],
                    ocp[hh][:, sl * 512:(sl + 1) * 512],
                    bcast)

            def s3_half(ci, mt, nt, act_copy=False):
                # out[ci*LQ+mt*128 : +128, nt*512 : +512]: contraction over
                # both pairs (kt) in one 2-matmul group + copy + store
                pso = psPp.tile([128, 512], f32, tag="psP")
                for kt in range(2):
                    nc.tensor.matmul(
                        pso[:, 0:512],
                        lhsT=oT[ci][:, kt, mt * 128:(mt + 1) * 128],
                        rhs=wo_sb[:, kt, nt * 512:(nt + 1) * 512],
                        start=(kt == 0), stop=(kt == 1),
                    )
                osb = opool.tile([128, 512], f32, tag="osb")
                if act_copy:
                    nc.scalar.copy(osb, pso)
                else:
                    nc.vector.tensor_copy(osb, pso)
                nc.sync.dma_start(
                    out[ci * LQ + mt * 128:ci * LQ + (mt + 1) * 128,
                        nt * 512:(nt + 1) * 512], osb)

            def s3_full(ci, mt, act_copy=False):
                # tail-only full-width piece through the (now idle) psS
                # pool: 2-deep pipelining instead of the serial psP chain
                pso = psSp.tile([128, 1024], f32, tag="psS")
                for kt in range(2):
                    for nt in range(2):
                        nc.tensor.matmul(
                            pso[:, nt * 512:(nt + 1) * 512],
                            lhsT=oT[ci][:, kt, mt * 128:(mt + 1) * 128],
                            rhs=wo_sb[:, kt, nt * 512:(nt + 1) * 512],
                            start=(kt == 0), stop=(kt == 1),
                        )
                osb = opool.tile([128, 1024], f32, tag="osb")
                if act_copy:
                    nc.scalar.copy(osb, pso)
                else:
                    nc.vector.tensor_copy(osb, pso)
                nc.sync.dma_start(
                    out[ci * LQ + mt * 128:ci * LQ + (mt + 1) * 128, :],
                    osb)

            # ---- hook schedule: per unit, slot -> list of thunks.  Each
            # psP group occupies 2 slots (group + copy through the single
            # buffer), so group starts are spaced 2 apart with deadlines:
            # V(j) copy by slot 2j (its O slot), K pieces by their S-emit.
            hooks = [dict() for _ in range(4)]

            def add_hook(ui, s, job):
                hooks[ui].setdefault(s, []).append(job)

            def pj(ct, lh, sl, w=None):
                w_sb, a_sb, d_sb = ((wk_sb, yT_sb, kT_sb) if w == "k"
                                    else (wq_sb, xT_sb, qT_sb))
                return lambda: proj_group(w_sb, a_sb, d_sb, ct, lh, sl)

            u0 = {1: pj(0, 0, 1, "k"),       # kT p0 lk 512-1023, S-emit s=6
                  3: lambda: v_group(8),
                  5: lambda: v_group(9),
                  7: lambda: v_group(10),
                  9: pj(0, 1, 0, "k"),       # kT p0 lk 1024-1535, emit s=14
                  11: lambda: v_group(11),
                  13: lambda: v_group(12),
                  15: lambda: v_group(13),
                  17: pj(0, 1, 1, "k"),      # kT p0 lk 1536-2047, emit s=22
                  19: lambda: v_group(14),
                  21: lambda: v_group(15),
                  25: pj(1, 0, 0),           # qT p1 q 0-511, due unit1 s=0
                  27: pj(1, 0, 0, "k"),      # kT p1 lk 0-511, due unit1 s=0
                  29: pj(1, 0, 1)}           # qT p1 q 512-1023, due u1 s=1
            u1 = {4: pj(1, 0, 1, "k"),       # kT p1 lk 512-1023, emit s=6
                  10: pj(1, 1, 0, "k"),
                  18: pj(1, 1, 1, "k"),
                  24: pj(0, 1, 0),           # qT p0 q 1024-1535, due u2 s=0
                  28: pj(0, 1, 1)}
            u2 = {8: pj(1, 1, 0),            # due unit3, plenty of slack
                  10: pj(1, 1, 1)}
            for ui, table in ((0, u0), (1, u1), (2, u2)):
                for s, job in table.items():
                    add_hook(ui, s, job)
            # chunk-0 stage 3: oT[0] final once unit-1's norm-b lands
            # (unit-2 slots 4-7); sl0 rows first, then sl1 rows
            for i, (mt, nt) in enumerate((m, n) for m in range(8)
                                         for n in range(2)):
                add_hook(2, 12 + i, lambda m=mt, n=nt: s3_half(0, m, n))

            # ---- startup: what slot 0 strictly needs runs through the
            # psS pool (no other user yet, so it pipelines 2-deep); the
            # first six V tiles ride the psP chain inside the DMA window --
            proj_group(wk_sb, yT_sb, kT_sb, 0, 0, 0, pool=psSp)
            proj_group(wq_sb, xT_sb, qT_sb, 0, 0, 0, pool=psSp)
            proj_group(wq_sb, xT_sb, qT_sb, 0, 0, 1, pool=psSp)
            for j in range(8):
                v_group(j)
            state = {(0, 0): emit_S(0, 0, 0, 0), (0, 1): emit_S(0, 0, 0, 1)}

            # ---- main loop: 4 units x 32 slots --------------------------
            psO_prev = psD_prev = None
            for ui in range(4):
                p, ci = UNITS[ui]
                psO_t = psOp.tile([128, 1024], f32, tag="psO")
                psD_t = psDp.tile([128, 512], f32, tag="psD")
                pts = {}
                for s in range(32):
                    ptt = ptpool.tile([128, 1024], bf16, tag="pt")
                    nc.scalar.activation(ptt, state.pop((ui, s)), Exp)
                    pts[s] = ptt
                    t = s + 2
                    if t < 32:
                        state[(ui, t)] = emit_S(p, ci, t // 2, t % 2)
                    elif ui + 1 < 4:
                        np_, nci = UNITS[ui + 1]
                        state[(ui + 1, t - 32)] = emit_S(
                            np_, nci, (t - 32) // 2, (t - 32) % 2)
                    if s >= 1:
                        emit_O(p, psO_t, pts[s - 1], (s - 1) // 2,
                               (s - 1) % 2)
                    # denominator quads lag 3 lk tiles so the previous
                    # unit's psDen drain (slots 0-3) finishes first
                    if s >= 6 and s % 2 == 0:
                        k = (s - 6) // 2
                        emit_den(psD_t, pts[2 * k], pts[2 * k + 1], k)
                        del pts[2 * k], pts[2 * k + 1]
                    elif s == 31:
                        emit_den(psD_t, pts[26], pts[27], 13)
                        del pts[26], pts[27]
                    # previous unit's normalize: psO evacuation is the
                    # only DVE work in slots 0-1 (it gates this unit's
                    # first O matmul); recips land in slots 2-3 (they gate
                    # the first denominator quad at slot 6); broadcast+mul
                    # pieces follow in slots 4-7
                    if psO_prev is not None and s == 0:
                        ocp = evac_O(psO_prev)
                        rcell = {}
                        for i, (hh, sl) in enumerate(
                                ((0, 0), (1, 0), (0, 1), (1, 1))):
                            add_hook(ui, 2 + i // 2,
                                     lambda pd=psD_prev, h=hh, ss=sl,
                                     rc=rcell: rc.__setitem__(
                                         (h, ss), norm_a(pd, h, ss)))
                            add_hook(ui, 4 + i,
                                     lambda u=ui - 1, oc=ocp, h=hh, ss=sl,
                                     rc=rcell:
                                     norm_b(u, rc[(h, ss)], oc, h, ss))
                    for job in hooks[ui].get(s, ()):
                        job()
                emit_O(p, psO_t, pts[31], NKT - 1, 1)
                emit_den(psD_t, pts[28], pts[29], 14)
                emit_den(psD_t, pts[30], pts[31], 15)
                psO_prev, psD_prev = psO_t, psD_t

            # ---- tail: unit-3 normalize + chunk-1 stage 3 (full-width
            # pieces, 2-deep through the now-idle psS pool, copies
            # alternating between ScalarE and VectorE) --------------------
            ocp = evac_O(psO_prev)
            for hh in range(2):
                norm_b(3, norm_a(psD_prev, hh, 0), ocp, hh, 0)
            for mt in range(4):
                s3_full(1, mt, act_copy=(mt % 2 == 0))
            for hh in range(2):
                norm_b(3, norm_a(psD_prev, hh, 1), ocp, hh, 1)
            for mt in range(4, 8):
                s3_full(1, mt, act_copy=(mt % 2 == 0))
    nc.compile()
    return nc


def _get_nc():
    if "nc" not in _CACHE:
        _CACHE["nc"] = _build()
    return _CACHE["nc"]


def _pack_pm(a, t):
    # [t*128, N] -> [128, t, N] partition-major
    return a.reshape(t, 128, -1).transpose(1, 0, 2)


def _pack_act(a):
    # x[b] [L, H] -> xT packed [128, NLQ(lh), 2(sl), 8(t), 512] bf16
    v = _pack_pm(np.ascontiguousarray(a.T), 8)          # [128, 8, L]
    v = v.reshape(128, 8, NLQ, 2, 512).transpose(0, 2, 3, 1, 4)
    return np.ascontiguousarray(v).astype(BF16)


def _in_maps(x, y, Wq, Wk, Wv, Wo):
    maps = []
    for core in range(8):
        b, g = core // GP, core % GP
        cs = slice(g * CH, (g + 1) * CH)
        maps.append({
            "xT": _pack_act(x[b]),
            "yT": _pack_act(y[b]),
            "wq": np.ascontiguousarray(
                _pack_pm(Wq[:, cs] * np.float32(0.125), 8)).astype(BF16),
            "wk": np.ascontiguousarray(_pack_pm(Wk[:, cs], 8)).astype(BF16),
            "wv": np.ascontiguousarray(_pack_pm(Wv[:, cs], 8)).astype(BF16),
            "wo": np.ascontiguousarray(_pack_pm(Wo[cs, :], 2)).astype(BF16),
        })
    return maps


def _install_ntff_hook():
    """Provide the antenv.axon_hooks shim missing from this container so
    run_bass_kernel_spmd(trace=True) can drive NTFF profiling via ctypes."""
    import sys
    import types
    try:
        from antenv.axon_hooks import get_axon_ntff_profile_hook  # noqa: F401
        return
    except ImportError:
        pass
    from trn_agent_boot.trn_boot import _ntff_profile_via_ctypes
    hook = _ntff_profile_via_ctypes("/opt/axon/libaxon_pjrt.so")
    mod = types.ModuleType("antenv.axon_hooks")
    mod.get_axon_ntff_profile_hook = lambda: hook
    mod.set_axon_ntff_profile_hook = lambda h: None
    sys.modules["antenv.axon_hooks"] = mod


def _run(inputs, trace=False):
    from concourse import bass_utils

    if trace:
        _install_ntff_hook()

    x, y, bias = inputs["x"], inputs["y"], inputs["bias"]
    if np.count_nonzero(np.asarray(bias)):
        raise NotImplementedError("nonzero attention bias not supported")
    nc = _get_nc()
    maps = _in_maps(np.asarray(x, np.float32), np.asarray(y, np.float32),
                    np.asarray(inputs["Wq"], np.float32),
                    np.asarray(inputs["Wk"], np.float32),
                    np.asarray(inputs["Wv"], np.float32),
                    np.asarray(inputs["Wo"], np.float32))
    res = bass_utils.run_bass_kernel_spmd(
        nc, maps, list(range(8)), trace=trace)
    out = np.zeros((B, L, H), np.float32)
    for core in range(8):
        out[core // GP] += res.results[core]["out"]
    return out, res


def kernel(**inputs):
    out, _ = _run(inputs, trace=False)
    return out
